# revision 15
# baseline (speedup 1.0000x reference)
"""Causal single-head attention (n=8192, d_model=1024, d_head=128) on 8 TRN2 cores.

v3 strategy (sequence-parallel queries; K/V part-replicated, part-gathered):
  - Queries: core c owns rows {8i + c : i in [0, 1024)} (mod-8 interleave).
    Causal work per row ~ q/128 tiles, so interleaving balances cores exactly
    and every core runs the *identical* instruction stream; per-core
    differences enter only through data (host-gathered xq + mask tiles).
  - K/V for chunks 0..M-1 (M = LOCAL_CHUNKS) are computed locally on every
    core: attention over early keys starts with zero communication latency.
    Chunks M..7 arrive via ONE AllGather of per-core own-chunk projections,
    triggered at ~15us and not consumed until the final diagonal groups —
    the collective's entry latency and mesh transfer hide under attention.
    M=8 disables the collective entirely (fully replicated projections).
  - All matmul operands are bf16 (PSUM accumulation stays f32): halves HBM
    and SBUF traffic, and the PE runs 1 cycle/row at any moving width.
  - Attention (transposed layout, no on-chip transposes in the loop):
       S^T[j, q]   = (K^T_J)^T Q^T    (lhsT = K^T 128-tile, rhs = Q^T cols)
       expS        = exp(S^T / sqrt(d))    (ACT, reads PSUM, writes SBUF)
       O^T[h, q]  += V_J^T expS       (lhsT = V j-tile,     rhs = expS)
       Z[q]       += ones^T gs        (gs = DVE sum of the group's 4 expS
                                       tiles -> 4x less PE time for Z)
       y[q, d]     = (O^T/Z)^T W_o    (normalize O^T by 1/Z, then project)
  - No softmax max-subtraction: scores are ~N(0,1) (|s| < ~8), exp is safe.
  - Causal masking: only j-tiles with j <= q are computed; the single
    diagonal tile per (J, q-window) is masked multiplicatively after exp.
  - y is stored bf16 and upconverted on the host.
"""

import numpy as np

N_CTX = 8192
D_MODEL = 1024
D_HEAD = 128
NCORES = 8
P = 128
KT = D_MODEL // P          # 8 k-tiles of the contraction dim
R = N_CTX // NCORES        # 1024 query rows per core (also rows per K/V chunk)
RCH = 512                  # row chunk for projection stages
NJ = N_CTX // P            # 64 key tiles
HALF = R // 2              # 512 query columns per attention pass
INV_SQRT_D = float(1.0 / np.sqrt(D_HEAD))
LOCAL_CHUNKS = 4           # chunks every core projects itself (8 = no AG)

_CACHE = {}


def _build(parts=frozenset({'s', 'pv', 'z', 'out'}), local_chunks=None):
    """Build + compile the SPMD program (one program, 8 cores)."""
    from contextlib import ExitStack

    import concourse.mybir as mybir
    import concourse.tile as tile
    from concourse import bacc

    M = LOCAL_CHUNKS if local_chunks is None else local_chunks
    use_ag = M < NCORES

    f32 = mybir.dt.float32
    f32r = mybir.dt.float32r
    bf16 = mybir.dt.bfloat16
    Exp = mybir.ActivationFunctionType.Exp

    nc = bacc.Bacc("TRN2", target_bir_lowering=False, debug=False,
                   num_devices=NCORES)

    # host pre-tiled inputs (see _host_in_maps)
    xtl = nc.dram_tensor("xtl", [2 * M, P, KT, RCH], bf16,
                         kind="ExternalInput")
    xq = nc.dram_tensor("xq", [2, P, KT, RCH], bf16, kind="ExternalInput")
    wq = nc.dram_tensor("wq", [P, KT, P], bf16, kind="ExternalInput")
    wk = nc.dram_tensor("wk", [P, KT, P], bf16, kind="ExternalInput")
    wv = nc.dram_tensor("wv", [P, KT, P], bf16, kind="ExternalInput")
    wo = nc.dram_tensor("wo", [D_HEAD, D_MODEL], bf16, kind="ExternalInput")
    masks = nc.dram_tensor("masks", [P, 8, P], bf16, kind="ExternalInput")
    eye = nc.dram_tensor("eye", [P, P], bf16, kind="ExternalInput")
    ones = nc.dram_tensor("ones", [P, 1], bf16, kind="ExternalInput")
    eyef = nc.dram_tensor("eyef", [P, P], f32, kind="ExternalInput")
    if use_ag:
        xtc = nc.dram_tensor("xtc", [2, P, KT, RCH], bf16,
                             kind="ExternalInput")
    y = nc.dram_tensor("y", [R, D_MODEL], bf16, kind="ExternalOutput")

    with tile.TileContext(nc) as tc, ExitStack() as ctx:
        consts = ctx.enter_context(tc.tile_pool(name="consts", bufs=1))
        xpool = ctx.enter_context(tc.tile_pool(name="xpool", bufs=5))
        xqpool = ctx.enter_context(tc.tile_pool(name="xqpool", bufs=2))
        xtcpool = ctx.enter_context(tc.tile_pool(name="xtcpool", bufs=2))
        vpool = ctx.enter_context(tc.tile_pool(name="vpool", bufs=2))
        ypool = ctx.enter_context(tc.tile_pool(name="ypool", bufs=4))
        sepool = ctx.enter_context(tc.tile_pool(name="sepool", bufs=4))
        gpool = ctx.enter_context(tc.tile_pool(name="gpool", bufs=2))
        ppj = ctx.enter_context(tc.tile_pool(name="ppj", bufs=2, space="PSUM"))
        pss = ctx.enter_context(tc.tile_pool(name="pss", bufs=2, space="PSUM"))
        pacc = ctx.enter_context(tc.tile_pool(name="pacc", bufs=1, space="PSUM"))
        if use_ag:
            dram = ctx.enter_context(tc.tile_pool(name="dram", bufs=1,
                                                  space="DRAM"))

        # ---- persistent SBUF ----
        wq_sb = consts.tile([P, KT, P], bf16, tag="wq")
        wk_sb = consts.tile([P, KT, P], bf16, tag="wk")
        wv_sb = consts.tile([P, KT, P], bf16, tag="wv")
        wo_sb = consts.tile([P, D_MODEL], bf16, tag="wo")
        masks_sb = consts.tile([P, 8, P], bf16, tag="masks")
        eye_sb = consts.tile([P, P], bf16, tag="eye")
        ones_sb = consts.tile([P, 1], bf16, tag="ones")
        eyef_sb = consts.tile([P, P], f32, tag="eyef")
        qT_sb = consts.tile([P, R], bf16, tag="qT")
        kT_sb = consts.tile([P, N_CTX], bf16, tag="kT")
        v_sb = consts.tile([P, NJ, P], bf16, tag="v")
        oTu_sb = consts.tile([P, R], bf16, tag="oTu")  # unnormalized O^T
        zrow_sb = consts.tile([1, HALF], f32, tag="zrow")
        zrec_sb = consts.tile([P, 8], f32, tag="zrec")  # 1/Z, q-tile cols
        if use_ag:
            kc_sb = consts.tile([P, 2, RCH], bf16, tag="kc")  # own-chunk K^T
            vc_sb = consts.tile([P, KT, P], bf16, tag="vc")   # own-chunk V
            ag_in = dram.tile([P, 4 * RCH], bf16, tag="agi", name="ag_in")
            ag_out = dram.tile([NCORES, P, 4 * RCH], bf16, tag="ago",
                               addr_space="Shared", name="ag_out")

        # PSUM accumulators: O^T and Z, one half live at a time (shared slots)
        acc = {}

        def get_acc(hf):
            if hf not in acc:
                acc[hf] = (pacc.tile([P, HALF], f32, tag="oT",
                                     name=f"oT_ps{hf}"),
                           pacc.tile([1, HALF], f32, tag="z",
                                     name=f"z_ps{hf}"))
            return acc[hf]

        def load_x(src, idx, name, pool=None, eng=None):
            x_t = (pool or xpool).tile([P, KT, RCH], bf16, tag="xt",
                                       name=name)
            (eng or nc.sync).dma_start(
                out=x_t.rearrange("p kt r -> p (kt r)"),
                in_=src[idx].rearrange("p kt r -> p (kt r)"))
            return x_t

        def proj_chain(w_sb, x_t, name):
            ps = ppj.tile([P, RCH], f32, tag="pj", name=name)
            for kt in range(KT):
                nc.tensor.matmul(ps, w_sb[:, kt, :], x_t[:, kt, :],
                                 start=(kt == 0), stop=(kt == KT - 1))
            return ps

        def kv_stage(x_t, k_dst, v_j0, tag):
            """Project one 512-row stage of x^T into K^T cols `k_dst` and
            V tiles v_j0..v_j0+3 (via PE transpose)."""
            kps = proj_chain(wk_sb, x_t, f"k_ps_{tag}")
            nc.vector.tensor_copy(k_dst, kps)
            vps = proj_chain(wv_sb, x_t, f"v_ps_{tag}")
            vt_tmp = vpool.tile([P, RCH], bf16, tag="vt", name="vt_tmp")
            nc.scalar.copy(vt_tmp, vps)
            for t in range(RCH // P):
                vtr = ppj.tile([P, P], bf16, tag="pj", name="vtr_ps")
                nc.tensor.transpose(vtr, vt_tmp[:, t * P:(t + 1) * P], eye_sb)
                yield t, vtr

        def local_stage(l, x_t):
            """Local projection of global rows [512l, 512l+512)."""
            for t, vtr in kv_stage(x_t, kT_sb[:, l * RCH:(l + 1) * RCH],
                                   4 * l, f"l{l}"):
                nc.vector.tensor_copy(v_sb[:, 4 * l + t, :], vtr)

        def own_stage(s, x_t):
            """Own-chunk projection stage s -> staging tiles for the AG."""
            for t, vtr in kv_stage(x_t, kc_sb[:, s, :], 0, f"o{s}"):
                nc.vector.tensor_copy(vc_sb[:, 4 * s + t, :], vtr)

        def bounce(s):
            """Own-chunk stage s -> HBM bounce buffer feeding the AG (on
            the scalar HWDGE queue: fast completion, engine idle here)."""
            nc.scalar.dma_start(out=ag_in[:, s * RCH:(s + 1) * RCH],
                                in_=kc_sb[:, s, :])
            nc.scalar.dma_start(
                out=ag_in[:, (2 + s) * RCH:(3 + s) * RCH],
                in_=vc_sb[:, 4 * s:4 * s + 4,
                          :].rearrange("p t d -> p (t d)"))

        def attention_group(hf, Js):
            """A uniform group of key-tiles Js against query-half hf.

            All Js must share the same query-column window. Structure:
            S matmuls into paired 2-bank PSUM tiles, one exp per pair,
            diagonal masks, batched PV matmuls, and one Z matmul over the
            DVE-summed group tile.
            """
            t0 = Js[0] // 8
            start_t = max(t0, 4 * hf)
            col0 = P * start_t              # global qT column
            w = HALF * (hf + 1) - col0      # 512/384/256/128
            lc0 = col0 - HALF * hf          # column inside this half's accums
            assert all(max(J // 8, 4 * hf) == start_t for J in Js)
            off1 = RCH if w > 256 else w    # second-in-pair region offset
            diag_grp = (t0 >= 4 * hf)
            regions = []                    # (J, se_tile, offset)
            for pi in range(0, len(Js), 2):
                pair = Js[pi:pi + 2]
                se = sepool.tile([P, 2 * RCH], bf16, tag="se", name="se")
                if 's' in parts:
                    s_ps = pss.tile([P, 2 * RCH], f32, tag="s", name="s_ps")
                    for idx, J in enumerate(pair):
                        off = idx * off1
                        nc.tensor.matmul(s_ps[:, off:off + w],
                                         kT_sb[:, J * P:(J + 1) * P],
                                         qT_sb[:, col0:col0 + w],
                                         start=True, stop=True)
                    if off1 == w or len(pair) == 1:   # contiguous regions
                        width = off1 * (len(pair) - 1) + w
                        nc.scalar.activation(se[:, :width], s_ps[:, :width],
                                             Exp, scale=INV_SQRT_D)
                    else:                             # gapped (w=384): split
                        for idx in range(len(pair)):
                            off = idx * off1
                            nc.scalar.activation(se[:, off:off + w],
                                                 s_ps[:, off:off + w],
                                                 Exp, scale=INV_SQRT_D)
                    if diag_grp:  # diagonal mask on first 128 cols
                        for idx, J in enumerate(pair):
                            off = idx * off1
                            nc.vector.tensor_mul(se[:, off:off + P],
                                                 se[:, off:off + P],
                                                 masks_sb[:, J % 8, :])
                for idx, J in enumerate(pair):
                    regions.append((J, se, idx * off1))
            oT_a, z_a = get_acc(hf)
            last_J = 32 * (hf + 1) - 1
            if 'pv' in parts:
                for (J, se, off) in regions:
                    nc.tensor.matmul(oT_a[:, lc0:lc0 + w], v_sb[:, J, :],
                                     se[:, off:off + w],
                                     start=(J == 0), stop=(J == last_J))
            if 'z' in parts:
                gs = gpool.tile([P, RCH], bf16, tag="gs", name="gs")
                (J0, se0, o0), (J1, se1, o1) = regions[0], regions[1]
                nc.vector.tensor_add(gs[:, :w], se0[:, o0:o0 + w],
                                     se1[:, o1:o1 + w])
                for (J, se, off) in regions[2:]:
                    nc.vector.tensor_add(gs[:, :w], gs[:, :w],
                                         se[:, off:off + w])
                nc.tensor.matmul(z_a[0:1, lc0:lc0 + w], ones_sb[:, 0:1],
                                 gs[:, :w],
                                 start=(Js[0] == 0), stop=(Js[-1] == last_J))

        def normalize(hf):
            """Stage unnormalized O^T to SBUF and build the 1/Z columns.

            Z lives as a [1, 512] PSUM row (queries on the free axis); PE
            transposes turn it into [128, 4] columns so the reciprocal runs
            128 lanes wide and y_project consumes it as a per-partition
            scale (the division folds into the y copy-out).
            """
            c0 = hf * HALF
            oT_a, z_a = get_acc(hf)
            # z row first and on ACT: the PE transposes unblock while the
            # DVE is still copying O^T out of PSUM
            nc.scalar.copy(zrow_sb[0:1, :], z_a[0:1, :])
            nc.vector.tensor_copy(oTu_sb[:, c0:c0 + HALF], oT_a)
            zt_ps = ppj.tile([P, 4], f32, tag="pj", name="zt_ps")
            for t in range(4):
                nc.tensor.transpose(zt_ps[:, t:t + 1],
                                    zrow_sb[0:1, t * P:(t + 1) * P],
                                    eyef_sb[0:1, 0:1])
            nc.vector.reciprocal(zrec_sb[:, 4 * hf:4 * hf + 4], zt_ps)

        def y_project(hf, i):
            """i-th (query-tile, d-chunk) output block of half hf; the copy
            out of PSUM alternates ACT/DVE so neither engine serializes the
            output pipeline."""
            qt = 4 * hf + i // 2
            dc = i % 2
            pool = pss if i % 2 == 0 else ppj
            y_ps = pool.tile([P, RCH], f32, tag="s" if i % 2 == 0 else "pj",
                             name="y_ps")
            nc.tensor.matmul(y_ps, oTu_sb[:, qt * P:(qt + 1) * P],
                             wo_sb[:, dc * RCH:(dc + 1) * RCH],
                             start=True, stop=True)
            y_sb = ypool.tile([P, RCH], bf16, tag="y", name="y_sb")
            if i % 2 == 0:
                nc.scalar.mul(y_sb, y_ps, zrec_sb[:, qt:qt + 1])
            else:
                nc.vector.tensor_scalar_mul(y_sb, y_ps, zrec_sb[:, qt:qt + 1])
            eng = nc.sync if hf == 1 else nc.gpsimd
            eng.dma_start(out=y[qt * P:(qt + 1) * P,
                                dc * RCH:(dc + 1) * RCH], in_=y_sb)

        # ---- emission ----
        # inputs spread over three DMA queues (sync + scalar HWDGE, gpsimd
        # SWDGE) — the first ~40us of attention consumes ~350GB/s of input,
        # near the per-core HBM ceiling, so all queues pull in parallel; the
        # own-chunk loads go last on sync — the AllGather can't start before
        # the slowest rank arrives anyway, so they are off the critical path
        nc.scalar.dma_start(out=masks_sb, in_=masks[:, :, :])
        nc.scalar.dma_start(out=ones_sb, in_=ones[:, :])
        nc.scalar.dma_start(out=eyef_sb, in_=eyef[:, :])
        nc.sync.dma_start(out=wk_sb, in_=wk[:, :, :])
        nc.sync.dma_start(out=wv_sb, in_=wv[:, :, :])
        nc.sync.dma_start(out=eye_sb, in_=eye[:, :])
        nc.sync.dma_start(out=wq_sb, in_=wq[:, :, :])
        xq_ts = [load_x(xq, 0, "xq_t0", pool=xqpool)]
        nc.scalar.dma_start(out=wo_sb, in_=wo[:, :])
        xq_ts += [load_x(xq, 1, "xq_t1", pool=xqpool, eng=nc.scalar)]
        xtl_ts = [load_x(xtl, l, f"xtl_t{l}",
                         eng=(nc.gpsimd if l % 2 else nc.sync))
                  for l in range(2)]
        # own-chunk loads ride sync right behind xtl0: the own-stage
        # projections sit between the first attention groups in the in-order
        # PE stream, so a late xtc arrival would stall everything
        xtc_ts = [load_x(xtc, s, f"xtc_t{s}", pool=xtcpool)
                  for s in range(2)] if use_ag else []
        xtl_ts += [load_x(xtl, l, f"xtl_t{l}",
                          eng=(nc.gpsimd if l % 2 else nc.sync))
                   for l in range(2, 2 * M)]

        # half 0 with local stages and q-tile finishes threaded between
        # groups; own-chunk projection for the AG slots in after group 1
        pending = list(range(2, 2 * M))
        local_stage(0, xtl_ts[0])
        qps0 = proj_chain(wq_sb, xq_ts[0], "q_ps0")
        nc.vector.tensor_copy(qT_sb[:, 0:RCH], qps0)
        local_stage(1, xtl_ts[1])
        for n in range(8):
            attention_group(0, list(range(4 * n, 4 * n + 4)))
            if n == 0 and use_ag:
                own_stage(0, xtc_ts[0])
                bounce(0)
            if n == 1 and use_ag:
                own_stage(1, xtc_ts[1])
                bounce(1)
                nc.gpsimd.collective_compute(
                    "AllGather", mybir.AluOpType.bypass,
                    replica_groups=[list(range(NCORES))],
                    ins=[ag_in[:]], outs=[ag_out[:]])
                for c in range(M, NCORES):
                    nc.sync.dma_start(
                        out=kT_sb[:, c * R:(c + 1) * R],
                        in_=ag_out[c, :, 0:2 * RCH])
                    nc.sync.dma_start(
                        out=v_sb[:, 8 * c:8 * c + 8,
                                 :].rearrange("p t d -> p (t d)"),
                        in_=ag_out[c, :, 2 * RCH:4 * RCH])
            if pending and n < 6:
                l = pending.pop(0)
                local_stage(l, xtl_ts[l])
        qps1 = proj_chain(wq_sb, xq_ts[1], "q_ps1")
        nc.vector.tensor_copy(qT_sb[:, RCH:R], qps1)
        if 'out' in parts:
            normalize(0)
        # half 1: backfill (J < 32, full 512-wide windows) then diagonal,
        # with half-0 output blocks threaded between groups
        hf1_groups = [list(range(j, j + 4)) for j in range(0, 32, 4)] + \
                     [list(range(4 * n, 4 * n + 4)) for n in range(8, 16)]
        for i, Js in enumerate(hf1_groups):
            attention_group(1, Js)
            if pending:
                l = pending.pop(0)
                local_stage(l, xtl_ts[l])
            if 'out' in parts and i % 2 == 1:
                y_project(0, i // 2)
        if 'out' in parts:
            normalize(1)
            for i in range(8):
                y_project(1, i)

    nc.compile()
    return nc


def _get_nc():
    if "nc" not in _CACHE:
        _CACHE["nc"] = _build()
    return _CACHE["nc"]


def _host_in_maps(x, W_q, W_k, W_v, W_o, local_chunks=None):
    import ml_dtypes
    bf = ml_dtypes.bfloat16
    M = LOCAL_CHUNKS if local_chunks is None else local_chunks
    x = np.asarray(x, dtype=np.float32)
    xT_bf = np.ascontiguousarray(x.T).astype(bf)         # [1024, 8192]

    def stages(cols):
        """[n_stages, p, kt, r] view of x^T column block."""
        n = cols.shape[1] // RCH
        return np.ascontiguousarray(
            cols.reshape(KT, P, n, RCH).transpose(2, 1, 0, 3))

    def wtile(w):
        return np.ascontiguousarray(
            np.asarray(w, np.float32).reshape(KT, P, D_HEAD)
            .transpose(1, 0, 2)).astype(bf)

    wq_t, wk_t, wv_t = wtile(W_q), wtile(W_k), wtile(W_v)
    wo_t = np.ascontiguousarray(np.asarray(W_o, np.float32)).astype(bf)
    eye = np.eye(P, dtype=np.float32).astype(bf)
    xtl = stages(xT_bf[:, 0:M * R])
    pp = np.arange(P)[:, None, None]
    uu = np.arange(8)[None, :, None]
    di = np.arange(P)[None, None, :]
    in_maps = []
    for c in range(NCORES):
        xq_c = stages(xT_bf[:, c::NCORES])
        mask_c = (8 * di + c >= 128 * uu + pp).astype(np.float32)
        im = {
            "xtl": xtl, "xq": xq_c,
            "wq": wq_t, "wk": wk_t, "wv": wv_t, "wo": wo_t,
            "masks": np.ascontiguousarray(mask_c).astype(bf),
            "eye": eye,
            "ones": np.ones((P, 1), np.float32).astype(bf),
            "eyef": np.eye(P, dtype=np.float32),
        }
        if M < NCORES:
            im["xtc"] = stages(xT_bf[:, R * c:R * (c + 1)])
        in_maps.append(im)
    return in_maps


def _run(x, W_q, W_k, W_v, W_o, trace=False):
    from concourse.bass_utils import run_bass_kernel_spmd
    nc = _get_nc()
    in_maps = _host_in_maps(x, W_q, W_k, W_v, W_o)
    res = run_bass_kernel_spmd(nc, in_maps, list(range(NCORES)), trace=trace)
    out = np.empty((N_CTX, D_MODEL), dtype=np.float32)
    for c in range(NCORES):
        out[c::NCORES] = np.asarray(res.results[c]["y"], dtype=np.float32)
    return out, res


def kernel(x, W_q, W_k, W_v, W_o):
    out, _ = _run(x, W_q, W_k, W_v, W_o, trace=False)
    return out


# revision 16
# speedup vs baseline: 1.0430x; 1.0430x over previous
"""Causal single-head attention (n=8192, d_model=1024, d_head=128) on 8 TRN2 cores.

v3 strategy (sequence-parallel queries; K/V part-replicated, part-gathered):
  - Queries: core c owns rows {8i + c : i in [0, 1024)} (mod-8 interleave).
    Causal work per row ~ q/128 tiles, so interleaving balances cores exactly
    and every core runs the *identical* instruction stream; per-core
    differences enter only through data (host-gathered xq + mask tiles).
  - K/V for chunks 0..M-1 (M = LOCAL_CHUNKS) are computed locally on every
    core: attention over early keys starts with zero communication latency.
    Chunks M..7 arrive via ONE AllGather of per-core own-chunk projections,
    triggered at ~15us and not consumed until the final diagonal groups —
    the collective's entry latency and mesh transfer hide under attention.
    M=8 disables the collective entirely (fully replicated projections).
  - All matmul operands are bf16 (PSUM accumulation stays f32): halves HBM
    and SBUF traffic, and the PE runs 1 cycle/row at any moving width.
  - Attention (transposed layout, no on-chip transposes in the loop):
       S^T[j, q]   = (K^T_J)^T Q^T    (lhsT = K^T 128-tile, rhs = Q^T cols)
       expS        = exp(S^T / sqrt(d))    (ACT, reads PSUM, writes SBUF)
       O^T[h, q]  += V_J^T expS       (lhsT = V j-tile,     rhs = expS)
       Z[q]       += ones^T gs        (gs = DVE sum of the group's 4 expS
                                       tiles -> 4x less PE time for Z)
       y[q, d]     = (O^T/Z)^T W_o    (normalize O^T by 1/Z, then project)
  - No softmax max-subtraction: scores are ~N(0,1) (|s| < ~8), exp is safe.
  - Causal masking: only j-tiles with j <= q are computed; the single
    diagonal tile per (J, q-window) is masked multiplicatively after exp.
  - y is stored bf16 and upconverted on the host.
"""

import numpy as np

N_CTX = 8192
D_MODEL = 1024
D_HEAD = 128
NCORES = 8
P = 128
KT = D_MODEL // P          # 8 k-tiles of the contraction dim
R = N_CTX // NCORES        # 1024 query rows per core (also rows per K/V chunk)
RCH = 512                  # row chunk for projection stages
NJ = N_CTX // P            # 64 key tiles
HALF = R // 2              # 512 query columns per attention pass
INV_SQRT_D = float(1.0 / np.sqrt(D_HEAD))
LOCAL_CHUNKS = 4           # chunks every core projects itself (8 = no AG)

_CACHE = {}


def _build(parts=frozenset({'s', 'pv', 'z', 'out'}), local_chunks=None):
    """Build + compile the SPMD program (one program, 8 cores)."""
    from contextlib import ExitStack

    import concourse.mybir as mybir
    import concourse.tile as tile
    from concourse import bacc

    M = LOCAL_CHUNKS if local_chunks is None else local_chunks
    use_ag = M < NCORES

    f32 = mybir.dt.float32
    f32r = mybir.dt.float32r
    bf16 = mybir.dt.bfloat16
    Exp = mybir.ActivationFunctionType.Exp

    nc = bacc.Bacc("TRN2", target_bir_lowering=False, debug=False,
                   num_devices=NCORES)

    # host pre-tiled inputs (see _host_in_maps)
    xtl = nc.dram_tensor("xtl", [2 * M, P, KT, RCH], bf16,
                         kind="ExternalInput")
    xq = nc.dram_tensor("xq", [2, P, KT, RCH], bf16, kind="ExternalInput")
    wq = nc.dram_tensor("wq", [P, KT, P], bf16, kind="ExternalInput")
    wk = nc.dram_tensor("wk", [P, KT, P], bf16, kind="ExternalInput")
    wv = nc.dram_tensor("wv", [P, KT, P], bf16, kind="ExternalInput")
    wo = nc.dram_tensor("wo", [D_HEAD, D_MODEL], bf16, kind="ExternalInput")
    masks = nc.dram_tensor("masks", [P, 8, P], bf16, kind="ExternalInput")
    eye = nc.dram_tensor("eye", [P, P], bf16, kind="ExternalInput")
    ones = nc.dram_tensor("ones", [P, 1], bf16, kind="ExternalInput")
    eyef = nc.dram_tensor("eyef", [P, P], f32, kind="ExternalInput")
    if use_ag:
        xtc = nc.dram_tensor("xtc", [2, P, KT, RCH], bf16,
                             kind="ExternalInput")
    y = nc.dram_tensor("y", [R, D_MODEL], bf16, kind="ExternalOutput")

    with tile.TileContext(nc) as tc, ExitStack() as ctx:
        consts = ctx.enter_context(tc.tile_pool(name="consts", bufs=1))
        xpool = ctx.enter_context(tc.tile_pool(name="xpool", bufs=5))
        xqpool = ctx.enter_context(tc.tile_pool(name="xqpool", bufs=2))
        xtcpool = ctx.enter_context(tc.tile_pool(name="xtcpool", bufs=2))
        vpool = ctx.enter_context(tc.tile_pool(name="vpool", bufs=2))
        ypool = ctx.enter_context(tc.tile_pool(name="ypool", bufs=4))
        sepool = ctx.enter_context(tc.tile_pool(name="sepool", bufs=4))
        gpool = ctx.enter_context(tc.tile_pool(name="gpool", bufs=2))
        ppj = ctx.enter_context(tc.tile_pool(name="ppj", bufs=2, space="PSUM"))
        pss = ctx.enter_context(tc.tile_pool(name="pss", bufs=2, space="PSUM"))
        pacc = ctx.enter_context(tc.tile_pool(name="pacc", bufs=1, space="PSUM"))
        if use_ag:
            dram = ctx.enter_context(tc.tile_pool(name="dram", bufs=1,
                                                  space="DRAM"))

        # ---- persistent SBUF ----
        wq_sb = consts.tile([P, KT, P], bf16, tag="wq")
        wk_sb = consts.tile([P, KT, P], bf16, tag="wk")
        wv_sb = consts.tile([P, KT, P], bf16, tag="wv")
        wo_sb = consts.tile([P, D_MODEL], bf16, tag="wo")
        masks_sb = consts.tile([P, 8, P], bf16, tag="masks")
        eye_sb = consts.tile([P, P], bf16, tag="eye")
        ones_sb = consts.tile([P, 1], bf16, tag="ones")
        eyef_sb = consts.tile([P, P], f32, tag="eyef")
        qT_sb = consts.tile([P, R], bf16, tag="qT")
        kT_sb = consts.tile([P, N_CTX], bf16, tag="kT")
        v_sb = consts.tile([P, NJ, P], bf16, tag="v")
        oTu_sb = consts.tile([P, R], bf16, tag="oTu")  # unnormalized O^T
        zrow_sb = consts.tile([1, HALF], f32, tag="zrow")
        zrec_sb = consts.tile([P, 8], f32, tag="zrec")  # 1/Z, q-tile cols
        if use_ag:
            kc_sb = consts.tile([P, 2, RCH], bf16, tag="kc")  # own-chunk K^T
            vc_sb = consts.tile([P, KT, P], bf16, tag="vc")   # own-chunk V
            ag_in = dram.tile([P, 4 * RCH], bf16, tag="agi", name="ag_in")
            ag_out = dram.tile([NCORES, P, 4 * RCH], bf16, tag="ago",
                               addr_space="Shared", name="ag_out")

        # PSUM accumulators: O^T and Z, one half live at a time (shared slots)
        acc = {}

        def get_acc(hf):
            if hf not in acc:
                acc[hf] = (pacc.tile([P, HALF], f32, tag="oT",
                                     name=f"oT_ps{hf}"),
                           pacc.tile([1, HALF], f32, tag="z",
                                     name=f"z_ps{hf}"))
            return acc[hf]

        def load_x(src, idx, name, pool=None, eng=None, pieces=4):
            """One DMA instruction lands on ONE DMA engine (~35GB/s), so a
            1MB tile must be split into pieces to engage several engines."""
            x_t = (pool or xpool).tile([P, KT, RCH], bf16, tag="xt",
                                       name=name)
            step = KT // pieces
            for j in range(0, KT, step):
                (eng or nc.sync).dma_start(
                    out=x_t[:, j:j + step, :].rearrange("p k r -> p (k r)"),
                    in_=src[idx, :, j:j + step,
                            :].rearrange("p k r -> p (k r)"))
            return x_t

        def proj_chain(w_sb, x_t, name):
            ps = ppj.tile([P, RCH], f32, tag="pj", name=name)
            for kt in range(KT):
                nc.tensor.matmul(ps, w_sb[:, kt, :], x_t[:, kt, :],
                                 start=(kt == 0), stop=(kt == KT - 1))
            return ps

        def kv_stage(x_t, k_dst, v_j0, tag):
            """Project one 512-row stage of x^T into K^T cols `k_dst` and
            V tiles v_j0..v_j0+3 (via PE transpose)."""
            kps = proj_chain(wk_sb, x_t, f"k_ps_{tag}")
            nc.vector.tensor_copy(k_dst, kps)
            vps = proj_chain(wv_sb, x_t, f"v_ps_{tag}")
            vt_tmp = vpool.tile([P, RCH], bf16, tag="vt", name="vt_tmp")
            nc.scalar.copy(vt_tmp, vps)
            for t in range(RCH // P):
                vtr = ppj.tile([P, P], bf16, tag="pj", name="vtr_ps")
                nc.tensor.transpose(vtr, vt_tmp[:, t * P:(t + 1) * P], eye_sb)
                yield t, vtr

        def local_stage(l, x_t):
            """Local projection of global rows [512l, 512l+512)."""
            for t, vtr in kv_stage(x_t, kT_sb[:, l * RCH:(l + 1) * RCH],
                                   4 * l, f"l{l}"):
                nc.vector.tensor_copy(v_sb[:, 4 * l + t, :], vtr)

        def own_stage(s, x_t):
            """Own-chunk projection stage s -> staging tiles for the AG."""
            for t, vtr in kv_stage(x_t, kc_sb[:, s, :], 0, f"o{s}"):
                nc.vector.tensor_copy(vc_sb[:, 4 * s + t, :], vtr)

        def bounce(s):
            """Own-chunk stage s -> HBM bounce buffer feeding the AG (on
            the scalar HWDGE queue: fast completion, engine idle here)."""
            nc.scalar.dma_start(out=ag_in[:, s * RCH:(s + 1) * RCH],
                                in_=kc_sb[:, s, :])
            nc.scalar.dma_start(
                out=ag_in[:, (2 + s) * RCH:(3 + s) * RCH],
                in_=vc_sb[:, 4 * s:4 * s + 4,
                          :].rearrange("p t d -> p (t d)"))

        def attention_group(hf, Js):
            """A uniform group of key-tiles Js against query-half hf.

            All Js must share the same query-column window. Structure:
            S matmuls into paired 2-bank PSUM tiles, one exp per pair,
            diagonal masks, batched PV matmuls, and one Z matmul over the
            DVE-summed group tile.
            """
            t0 = Js[0] // 8
            start_t = max(t0, 4 * hf)
            col0 = P * start_t              # global qT column
            w = HALF * (hf + 1) - col0      # 512/384/256/128
            lc0 = col0 - HALF * hf          # column inside this half's accums
            assert all(max(J // 8, 4 * hf) == start_t for J in Js)
            off1 = RCH if w > 256 else w    # second-in-pair region offset
            diag_grp = (t0 >= 4 * hf)
            regions = []                    # (J, se_tile, offset)
            for pi in range(0, len(Js), 2):
                pair = Js[pi:pi + 2]
                se = sepool.tile([P, 2 * RCH], bf16, tag="se", name="se")
                if 's' in parts:
                    s_ps = pss.tile([P, 2 * RCH], f32, tag="s", name="s_ps")
                    for idx, J in enumerate(pair):
                        off = idx * off1
                        nc.tensor.matmul(s_ps[:, off:off + w],
                                         kT_sb[:, J * P:(J + 1) * P],
                                         qT_sb[:, col0:col0 + w],
                                         start=True, stop=True)
                    if off1 == w or len(pair) == 1:   # contiguous regions
                        width = off1 * (len(pair) - 1) + w
                        nc.scalar.activation(se[:, :width], s_ps[:, :width],
                                             Exp, scale=INV_SQRT_D)
                    else:                             # gapped (w=384): split
                        for idx in range(len(pair)):
                            off = idx * off1
                            nc.scalar.activation(se[:, off:off + w],
                                                 s_ps[:, off:off + w],
                                                 Exp, scale=INV_SQRT_D)
                    if diag_grp:  # diagonal mask on first 128 cols
                        for idx, J in enumerate(pair):
                            off = idx * off1
                            nc.vector.tensor_mul(se[:, off:off + P],
                                                 se[:, off:off + P],
                                                 masks_sb[:, J % 8, :])
                for idx, J in enumerate(pair):
                    regions.append((J, se, idx * off1))
            oT_a, z_a = get_acc(hf)
            last_J = 32 * (hf + 1) - 1
            if 'pv' in parts:
                for (J, se, off) in regions:
                    nc.tensor.matmul(oT_a[:, lc0:lc0 + w], v_sb[:, J, :],
                                     se[:, off:off + w],
                                     start=(J == 0), stop=(J == last_J))
            if 'z' in parts:
                gs = gpool.tile([P, RCH], bf16, tag="gs", name="gs")
                (J0, se0, o0), (J1, se1, o1) = regions[0], regions[1]
                nc.vector.tensor_add(gs[:, :w], se0[:, o0:o0 + w],
                                     se1[:, o1:o1 + w])
                for (J, se, off) in regions[2:]:
                    nc.vector.tensor_add(gs[:, :w], gs[:, :w],
                                         se[:, off:off + w])
                nc.tensor.matmul(z_a[0:1, lc0:lc0 + w], ones_sb[:, 0:1],
                                 gs[:, :w],
                                 start=(Js[0] == 0), stop=(Js[-1] == last_J))

        def normalize(hf):
            """Stage unnormalized O^T to SBUF and build the 1/Z columns.

            Z lives as a [1, 512] PSUM row (queries on the free axis); PE
            transposes turn it into [128, 4] columns so the reciprocal runs
            128 lanes wide and y_project consumes it as a per-partition
            scale (the division folds into the y copy-out).
            """
            c0 = hf * HALF
            oT_a, z_a = get_acc(hf)
            # z row first and on ACT: the PE transposes unblock while the
            # DVE is still copying O^T out of PSUM
            nc.scalar.copy(zrow_sb[0:1, :], z_a[0:1, :])
            nc.vector.tensor_copy(oTu_sb[:, c0:c0 + HALF], oT_a)
            zt_ps = ppj.tile([P, 4], f32, tag="pj", name="zt_ps")
            for t in range(4):
                nc.tensor.transpose(zt_ps[:, t:t + 1],
                                    zrow_sb[0:1, t * P:(t + 1) * P],
                                    eyef_sb[0:1, 0:1])
            nc.vector.reciprocal(zrec_sb[:, 4 * hf:4 * hf + 4], zt_ps)

        def y_project(hf, i):
            """i-th (query-tile, d-chunk) output block of half hf; the copy
            out of PSUM alternates ACT/DVE so neither engine serializes the
            output pipeline."""
            qt = 4 * hf + i // 2
            dc = i % 2
            pool = pss if i % 2 == 0 else ppj
            y_ps = pool.tile([P, RCH], f32, tag="s" if i % 2 == 0 else "pj",
                             name="y_ps")
            nc.tensor.matmul(y_ps, oTu_sb[:, qt * P:(qt + 1) * P],
                             wo_sb[:, dc * RCH:(dc + 1) * RCH],
                             start=True, stop=True)
            y_sb = ypool.tile([P, RCH], bf16, tag="y", name="y_sb")
            if i % 2 == 0:
                nc.scalar.mul(y_sb, y_ps, zrec_sb[:, qt:qt + 1])
            else:
                nc.vector.tensor_scalar_mul(y_sb, y_ps, zrec_sb[:, qt:qt + 1])
            eng = nc.sync if hf == 1 else nc.gpsimd
            eng.dma_start(out=y[qt * P:(qt + 1) * P,
                                dc * RCH:(dc + 1) * RCH], in_=y_sb)

        # ---- emission ----
        # inputs spread over three DMA queues (sync + scalar HWDGE, gpsimd
        # SWDGE) — the first ~40us of attention consumes ~350GB/s of input,
        # near the per-core HBM ceiling, so all queues pull in parallel; the
        # own-chunk loads go last on sync — the AllGather can't start before
        # the slowest rank arrives anyway, so they are off the critical path
        nc.scalar.dma_start(out=masks_sb, in_=masks[:, :, :])
        nc.scalar.dma_start(out=ones_sb, in_=ones[:, :])
        nc.scalar.dma_start(out=eyef_sb, in_=eyef[:, :])
        nc.sync.dma_start(out=wk_sb, in_=wk[:, :, :])
        nc.sync.dma_start(out=wv_sb, in_=wv[:, :, :])
        nc.sync.dma_start(out=eye_sb, in_=eye[:, :])
        nc.sync.dma_start(out=wq_sb, in_=wq[:, :, :])
        xq_ts = [load_x(xq, 0, "xq_t0", pool=xqpool)]
        nc.scalar.dma_start(out=wo_sb, in_=wo[:, :])
        xq_ts += [load_x(xq, 1, "xq_t1", pool=xqpool, eng=nc.scalar)]
        xtl_ts = [load_x(xtl, l, f"xtl_t{l}",
                         eng=(nc.gpsimd if l % 2 else nc.sync))
                  for l in range(2)]
        # own-chunk loads ride sync right behind xtl0: the own-stage
        # projections sit between the first attention groups in the in-order
        # PE stream, so a late xtc arrival would stall everything
        xtc_ts = [load_x(xtc, s, f"xtc_t{s}", pool=xtcpool)
                  for s in range(2)] if use_ag else []
        xtl_ts += [load_x(xtl, l, f"xtl_t{l}",
                          eng=(nc.gpsimd if l % 2 else nc.sync))
                   for l in range(2, 2 * M)]

        # half 0 with local stages and q-tile finishes threaded between
        # groups; own-chunk projection for the AG slots in after group 1
        pending = list(range(2, 2 * M))
        local_stage(0, xtl_ts[0])
        qps0 = proj_chain(wq_sb, xq_ts[0], "q_ps0")
        nc.vector.tensor_copy(qT_sb[:, 0:RCH], qps0)
        local_stage(1, xtl_ts[1])
        for n in range(8):
            attention_group(0, list(range(4 * n, 4 * n + 4)))
            if n == 0 and use_ag:
                own_stage(0, xtc_ts[0])
                bounce(0)
            if n == 1 and use_ag:
                own_stage(1, xtc_ts[1])
                bounce(1)
                nc.gpsimd.collective_compute(
                    "AllGather", mybir.AluOpType.bypass,
                    replica_groups=[list(range(NCORES))],
                    ins=[ag_in[:]], outs=[ag_out[:]])
                for c in range(M, NCORES):
                    for h in range(2):
                        nc.sync.dma_start(
                            out=kT_sb[:, c * R + h * RCH:
                                      c * R + (h + 1) * RCH],
                            in_=ag_out[c, :, h * RCH:(h + 1) * RCH])
                        nc.sync.dma_start(
                            out=v_sb[:, 8 * c + 4 * h:8 * c + 4 * h + 4,
                                     :].rearrange("p t d -> p (t d)"),
                            in_=ag_out[c, :, (2 + h) * RCH:(3 + h) * RCH])
            if pending and n < 6:
                l = pending.pop(0)
                local_stage(l, xtl_ts[l])
        qps1 = proj_chain(wq_sb, xq_ts[1], "q_ps1")
        nc.vector.tensor_copy(qT_sb[:, RCH:R], qps1)
        if 'out' in parts:
            normalize(0)
        # half 1: backfill (J < 32, full 512-wide windows) then diagonal,
        # with half-0 output blocks threaded between groups
        hf1_groups = [list(range(j, j + 4)) for j in range(0, 32, 4)] + \
                     [list(range(4 * n, 4 * n + 4)) for n in range(8, 16)]
        for i, Js in enumerate(hf1_groups):
            attention_group(1, Js)
            if pending:
                l = pending.pop(0)
                local_stage(l, xtl_ts[l])
            if 'out' in parts and i % 2 == 1:
                y_project(0, i // 2)
        if 'out' in parts:
            normalize(1)
            for i in range(8):
                y_project(1, i)

    nc.compile()
    return nc


def _get_nc():
    if "nc" not in _CACHE:
        _CACHE["nc"] = _build()
    return _CACHE["nc"]


def _host_in_maps(x, W_q, W_k, W_v, W_o, local_chunks=None):
    import ml_dtypes
    bf = ml_dtypes.bfloat16
    M = LOCAL_CHUNKS if local_chunks is None else local_chunks
    x = np.asarray(x, dtype=np.float32)
    xT_bf = np.ascontiguousarray(x.T).astype(bf)         # [1024, 8192]

    def stages(cols):
        """[n_stages, p, kt, r] view of x^T column block."""
        n = cols.shape[1] // RCH
        return np.ascontiguousarray(
            cols.reshape(KT, P, n, RCH).transpose(2, 1, 0, 3))

    def wtile(w):
        return np.ascontiguousarray(
            np.asarray(w, np.float32).reshape(KT, P, D_HEAD)
            .transpose(1, 0, 2)).astype(bf)

    wq_t, wk_t, wv_t = wtile(W_q), wtile(W_k), wtile(W_v)
    wo_t = np.ascontiguousarray(np.asarray(W_o, np.float32)).astype(bf)
    eye = np.eye(P, dtype=np.float32).astype(bf)
    xtl = stages(xT_bf[:, 0:M * R])
    pp = np.arange(P)[:, None, None]
    uu = np.arange(8)[None, :, None]
    di = np.arange(P)[None, None, :]
    in_maps = []
    for c in range(NCORES):
        xq_c = stages(xT_bf[:, c::NCORES])
        mask_c = (8 * di + c >= 128 * uu + pp).astype(np.float32)
        im = {
            "xtl": xtl, "xq": xq_c,
            "wq": wq_t, "wk": wk_t, "wv": wv_t, "wo": wo_t,
            "masks": np.ascontiguousarray(mask_c).astype(bf),
            "eye": eye,
            "ones": np.ones((P, 1), np.float32).astype(bf),
            "eyef": np.eye(P, dtype=np.float32),
        }
        if M < NCORES:
            im["xtc"] = stages(xT_bf[:, R * c:R * (c + 1)])
        in_maps.append(im)
    return in_maps


def _run(x, W_q, W_k, W_v, W_o, trace=False):
    from concourse.bass_utils import run_bass_kernel_spmd
    nc = _get_nc()
    in_maps = _host_in_maps(x, W_q, W_k, W_v, W_o)
    res = run_bass_kernel_spmd(nc, in_maps, list(range(NCORES)), trace=trace)
    out = np.empty((N_CTX, D_MODEL), dtype=np.float32)
    for c in range(NCORES):
        out[c::NCORES] = np.asarray(res.results[c]["y"], dtype=np.float32)
    return out, res


def kernel(x, W_q, W_k, W_v, W_o):
    out, _ = _run(x, W_q, W_k, W_v, W_o, trace=False)
    return out
